# revision 1
# baseline (speedup 1.0000x reference)
"""Memristor forward (nn_Memristor_78030965833729) — TRN2 Bass kernel, 8 cores.

Contract: kernel(Vin: np.ndarray[16,1024,1024] f32) -> np.ndarray[16,1024,1024] f32.

Sharding: channels split 8 ways (128 per core); batch and time whole per
core.  Per-core SBUF layout [128 part = channel, free = t*16 + b].  The
time recurrence runs per-step on [128,16] tiles carrying (fil, res, S);
the output current is computed in a vectorized per-block pass from the
stored per-step states.  Self-contained: no imports from this directory
besides the concourse runtime that ships with the container.
"""
import math

import numpy as np

import concourse.bass as bass
import concourse.mybir as mybir
import concourse.tile as tile
from concourse.bass_utils import run_bass_kernel_spmd
from concourse.dve_ops import RECIPROCAL_APPROX_NR as _RECIP_NR

F32 = mybir.dt.float32
AF = mybir.ActivationFunctionType
OP = mybir.AluOpType

# --- model constants (deterministic Memristor config) ---
DT = 0.001
G1DT = 0.6
G2DT = 0.002
G3DT = 0.005
MUDT = 0.22
BDT = 0.01
SM_THR = 0.999999
CM_THR = 1.000001
B_E1 = 1.0 + math.log(DT)
B_E2 = -1.0 + math.log(G3DT)
DENOM = float(np.float32(np.exp(np.float32(5.0))) - np.float32(1.0))
K = 1.0e12 / DENOM
B_E3 = 5.0 + math.log(K)

B_, T_, C_ = 16, 1024, 1024
NCORES = 8
PERC = C_ // NCORES  # 128 channels per core


def _split_excess_waits(nc) -> int:
    """TPB instructions encode at most 1 sync-wait (2 for EventSemaphore).
    Tile attaches all waits to the consumer; spill the excess into
    standalone EventSemaphore instructions on the same engine queue."""
    n_split = 0
    ctr = [0]

    def fresh_name() -> str:
        ctr[0] += 1
        return f"WSPLIT-{ctr[0]}"

    for f in nc.m.functions:
        for blk in f.blocks:
            insts = blk.instructions
            out = []
            changed = False
            for inst in insts:
                si = inst.sync_info
                waits = list(si.on_wait) if si is not None and si.on_wait else []
                cap = 2 if isinstance(inst, mybir.InstEventSemaphore) else 1
                if len(waits) <= cap:
                    out.append(inst)
                    continue
                changed = True
                keep = waits[:cap]
                extra = waits[cap:]
                for i in range(0, len(extra), 2):
                    ev = mybir.InstEventSemaphore(
                        name=fresh_name(),
                        engine=inst.engine,
                        ins=[],
                        outs=[],
                        sync_info=mybir.SyncInfo(on_wait=extra[i:i + 2],
                                                 on_update=[]),
                    )
                    out.append(ev)
                    n_split += 1
                inst.sync_info = mybir.SyncInfo(
                    on_wait=keep,
                    on_update=list(si.on_update) if si.on_update else [],
                )
                out.append(inst)
            if changed:
                blk.instructions = out
    return n_split


def build_kernel(T: int = T_, TB: int = 128):
    assert T % TB == 0
    NB = T // TB
    P, BATCH = 128, B_
    NF = T * BATCH
    W = BATCH

    nc = bass.Bass("TRN2", target_bir_lowering=False, debug=False)
    x = nc.dram_tensor("vin", [P, NF], F32, kind="ExternalInput")
    y = nc.dram_tensor("cur", [P, NF], F32, kind="ExternalOutput")

    for val in (B_E1, B_E2, 1.01, B_E3):
        t = nc.alloc_sbuf_tensor(f"cst-{val}", [128, 1], F32)
        nc.gpsimd.memset(t.ap(), val)
        nc.const_aps.aps[(F32, val)] = t.ap()
    nc.all_engine_barrier()

    with tile.TileContext(nc) as tc:
        with tc.tile_pool(name="io", bufs=1) as io_pool, \
             tc.tile_pool(name="state", bufs=2) as st_pool, \
             tc.tile_pool(name="tmp", bufs=4) as tp, \
             tc.tile_pool(name="p2", bufs=1) as p2, \
             tc.tile_pool(name="curp", bufs=2) as curp:
            vin = io_pool.tile([P, NF], F32, name="vin_sb")
            NCH = max(1, NF // 2048)
            csz = NF // NCH
            for c in range(NCH):
                nc.gpsimd.dma_start(vin[:, c * csz:(c + 1) * csz],
                                    x[:, c * csz:(c + 1) * csz])

            prev = None
            for blk in range(NB):
                Sb = st_pool.tile([P, (TB + 1) * W], F32, tag="Sb", name="Sb")
                Fb = st_pool.tile([P, (TB + 1) * W], F32, tag="Fb", name="Fb")
                Rb = st_pool.tile([P, (TB + 1) * W], F32, tag="Rb", name="Rb")
                if prev is None:
                    nc.vector.memset(Sb[:, 0:W], 0.0)
                    nc.vector.memset(Fb[:, 0:W], 0.0)
                    nc.vector.memset(Rb[:, 0:W], 0.0)
                else:
                    pS, pF, pR = prev
                    nc.vector.tensor_copy(Sb[:, 0:W], pS[:, TB * W:(TB + 1) * W])
                    nc.vector.tensor_copy(Fb[:, 0:W], pF[:, TB * W:(TB + 1) * W])
                    nc.vector.tensor_copy(Rb[:, 0:W], pR[:, TB * W:(TB + 1) * W])
                prev = (Sb, Fb, Rb)

                for s in range(TB):
                    t = blk * TB + s
                    V = vin[:, t * W:(t + 1) * W]
                    S0 = Sb[:, s * W:(s + 1) * W]
                    F0 = Fb[:, s * W:(s + 1) * W]
                    R0 = Rb[:, s * W:(s + 1) * W]
                    S1 = Sb[:, (s + 1) * W:(s + 2) * W]
                    F1o = Fb[:, (s + 1) * W:(s + 2) * W]
                    R1o = Rb[:, (s + 1) * W:(s + 2) * W]

                    def tt(name):
                        return tp.tile([P, W], F32, tag=name, name=name)

                    tot = tt("tot")
                    nc.vector.tensor_tensor(tot[:], R0, F0, OP.add)
                    nc.vector.tensor_scalar(tot[:], tot[:], 0.0, 1.0, OP.max, OP.min)
                    E1 = tt("E1")
                    nc.scalar.activation(E1[:], S0, AF.Exp, bias=B_E1, scale=-1.0)
                    E2 = tt("E2")
                    nc.scalar.activation(E2[:], S0, AF.Exp, bias=B_E2, scale=1.0)
                    LD = tt("LD")
                    nc.scalar.activation(LD[:], tot[:], AF.Ln, bias=1.01, scale=-1.0)
                    RD = tt("RD")
                    nc.scalar.activation(RD[:], LD[:], AF.Exp, bias=0.0, scale=-1.0)
                    VP = tt("VP")
                    nc.vector.tensor_scalar(VP[:], V, 0.0, None, OP.max)
                    PP = tt("PP")
                    nc.vector.tensor_tensor(PP[:], VP[:], E1[:], OP.mult)
                    DS = tt("DS")
                    nc.vector.tensor_tensor(DS[:], PP[:], E2[:], OP.subtract)
                    SM = tt("SM")
                    nc.vector.tensor_scalar(SM[:], tot[:], SM_THR, None, OP.is_gt)
                    DSM = tt("DSM")
                    nc.vector.tensor_tensor(DSM[:], DS[:], SM[:], OP.mult)
                    SA = tt("SA")
                    nc.vector.tensor_tensor(SA[:], S0, DSM[:], OP.add)
                    nc.vector.tensor_scalar(S1, SA[:], 1.0, None, OP.max)
                    CM = tt("CM")
                    nc.vector.tensor_scalar(CM[:], S0, CM_THR, None, OP.is_lt)
                    CF = tt("CF")
                    nc.vector.tensor_scalar(CF[:], CM[:], -G1DT, 1.0, OP.mult, OP.add)
                    CR = tt("CR")
                    nc.vector.tensor_scalar(CR[:], CM[:], -G2DT, 1.0, OP.mult, OP.add)
                    FF1 = tt("FF1")
                    nc.vector.tensor_tensor(FF1[:], F0, CF[:], OP.mult)
                    RR1 = tt("RR1")
                    nc.vector.tensor_tensor(RR1[:], R0, CR[:], OP.mult)
                    VP22 = tt("VP22")
                    nc.vector.tensor_scalar(VP22[:], V, 0.0, MUDT, OP.max, OP.mult)
                    W0 = tt("W0")
                    nc.vector.tensor_tensor(W0[:], VP22[:], RD[:], OP.mult)
                    WM = tt("WM")
                    nc.vector.tensor_tensor(WM[:], W0[:], CM[:], OP.mult)
                    CAP = tt("CAP")
                    nc.vector.tensor_tensor(CAP[:], FF1[:], RR1[:], OP.add)
                    nc.vector.tensor_scalar(CAP[:], CAP[:], -1.0, 1.0, OP.mult, OP.add)
                    DR = tt("DR")
                    nc.vector.tensor_tensor(DR[:], WM[:], CAP[:], OP.min)
                    F2 = tt("F2")
                    nc.vector.tensor_tensor(F2[:], FF1[:], DR[:], OP.add)
                    TF = tt("TF")
                    nc.vector.tensor_scalar(TF[:], F2[:], BDT, None, OP.mult)
                    B1 = tt("B1")
                    nc.vector.tensor_scalar(B1[:], RR1[:], -1.0, 1.0, OP.mult, OP.add)
                    TRp = tt("TRp")
                    nc.vector.tensor_tensor(TRp[:], TF[:], B1[:], OP.min)
                    TR = tt("TR")
                    nc.vector.tensor_tensor(TR[:], TRp[:], CM[:], OP.mult)
                    nc.vector.tensor_tensor(F1o, F2[:], TR[:], OP.subtract)
                    nc.vector.tensor_tensor(R1o, RR1[:], TR[:], OP.add)

                NB2 = TB * W
                Sv = Sb[:, W:(TB + 1) * W]
                Fv = Fb[:, W:(TB + 1) * W]
                Rv = Rb[:, W:(TB + 1) * W]
                Vv = vin[:, blk * NB2:(blk + 1) * NB2]
                Cv = curp.tile([P, NB2], F32, tag="Cv", name="Cv")

                def t2(name):
                    return p2.tile([P, NB2], F32, tag=name, name=name)

                ta, tb, tc2, td = t2("p2a"), t2("p2b"), t2("p2c"), t2("p2d")
                nc.vector.tensor_tensor(ta[:], Fv, Rv, OP.add)
                nc.vector.tensor_scalar(ta[:], ta[:], 0.0, 1.0, OP.max, OP.min)
                nc.scalar.activation(tb[:], ta[:], AF.Exp, bias=B_E3, scale=-5.0)
                nc.vector.tensor_scalar(tb[:], tb[:], K, None, OP.subtract)
                nc.vector.tensor_tensor(tc2[:], Sv, Sv, OP.mult)
                nc.vector.tensor_tensor(tb[:], tb[:], tc2[:], OP.mult)
                nc.vector.tensor_scalar(ta[:], ta[:], 1.0e7, None, OP.mult)
                nc.vector.tensor_tensor(tb[:], tb[:], ta[:], OP.add)
                nc.vector.reciprocal_approx_fast(td[:], tb[:])
                nc.vector._custom_dve(_RECIP_NR, out=td[:], in0=tb[:],
                                      in1=td[:], s0=2.0)
                nc.vector.tensor_tensor(tc2[:], Vv, tc2[:], OP.mult)
                nc.vector.tensor_tensor(Cv[:], tc2[:], td[:], OP.mult)
                nc.gpsimd.dma_start(y[:, blk * NB2:(blk + 1) * NB2], Cv[:])

    _split_excess_waits(nc)
    from concourse.library_overlay import lower_extended_insts
    lower_extended_insts(nc)
    return nc


_NC_CACHE = {}


def kernel(Vin: np.ndarray, _trace: bool = False):
    assert Vin.shape == (B_, T_, C_), Vin.shape
    Vin = np.ascontiguousarray(Vin, dtype=np.float32)

    if "nc" not in _NC_CACHE:
        _NC_CACHE["nc"] = build_kernel()
    nc = _NC_CACHE["nc"]

    # pack: per-core [128, T*B], channel-major partitions, free = t*16+b
    in_maps = []
    for k in range(NCORES):
        s = Vin[:, :, k * PERC:(k + 1) * PERC]          # [B,T,128]
        s = np.ascontiguousarray(np.transpose(s, (2, 1, 0)))  # [128,T,B]
        in_maps.append({"vin": s.reshape(PERC, T_ * B_)})

    res = run_bass_kernel_spmd(nc, in_maps, core_ids=list(range(NCORES)),
                               trace=_trace)

    out = np.empty((B_, T_, C_), dtype=np.float32)
    for k in range(NCORES):
        s = res.results[k]["cur"].reshape(PERC, T_, B_)
        out[:, :, k * PERC:(k + 1) * PERC] = np.transpose(s, (2, 1, 0))
    if _trace:
        return out, res
    return out
